# revision 29
# baseline (speedup 1.0000x reference)
"""Multi-headed attention TRN2 Bass kernel (v2).

Problem: B=2, S=2048, D=1024, H=16 heads (dh=64), fp32 in/out, bool mask.

Sharding (8 cores): data-parallel over B (2) x tensor-parallel over heads
(4 heads / 256 features per core). Each core computes its head-group's
q/k/v projections, masked softmax attention, and a partial output
projection (Wo columns for its heads). Host sums the 4 partials per batch
element (the TP all-reduce) and adds the bias.

v2 changes vs v1:
  - ONE input tensor ("blob") and ONE output per core. The per-call
    dispatch overhead of the axon/PJRT path scales ~60us per sharded
    buffer per exec; 7 buffers -> 2 buffers roughly halves the measured
    wall time before any device-side change.
  - fp16 everywhere (x, weights, keep-mask, output partials): half the
    DMA bytes, DVE 2x/4x fast modes, 1 cycle/row matmuls.
  - exp on ACT over [128, 1024] PSUM spans (two banks / both heads of a
    pair per call) to amortize the ~352-cycle ACT fixed overhead.
  - all mask multiplies on DVE (fp16 4x mode); prefetch DMAs issued from
    the idle gpsimd queue; emission order overlaps k/q0 projection fill
    with attention and runs the output projection per-qc instead of at
    the end.

Per-core layout (all fp16, feature/key-position major a.k.a. transposed):
  blob cols: [x: NQC*NDT*SC][keep: NQC*NKT*SC][wq|wk|wv: NDT*FPC][wo: 2*D]
  out: [P, NFT=8, NQC, SC] partial output (summed across 4 cores on host).

No max-subtraction in softmax: scores ~N(0,1), exp in fp32->fp16 is
plenty within the 2e-2 gate (measured ~5e-4 end to end).
"""

import math
from contextlib import ExitStack

import numpy as np

import concourse.mybir as mybir
import concourse.tile as tile
from concourse import bacc
from concourse.bass_utils import run_bass_kernel_spmd

B, S, D, H = 2, 2048, 1024, 16
DH = D // H                 # 64
NCORES = 8
GROUPS = NCORES // B        # 4 head-groups per batch element
FPC = D // GROUPS           # 256 features (4 heads) per core
P = 128
SC = 512                    # q/s chunk (free dim of most matmuls)
NQC = S // SC               # 4
NKT = S // P                # 16 k-position tiles
NDT = D // P                # 8 contraction tiles over D
NFT = D // P                # 8 output-feature tiles

# blob column offsets (fp16 elements per partition)
XW = NDT * SC               # 4096 cols per x chunk
KW = NKT * SC               # 8192 cols per keep chunk
WW = NDT * FPC              # 2048 cols per qkv weight
OW = (FPC // P) * D         # 2048 cols for wo
XOFF = 0
KOFF = XOFF + NQC * XW      # 16384
WQOFF = KOFF + NQC * KW     # 49152
WKOFF = WQOFF + WW
WVOFF = WKOFF + WW
WOOFF = WVOFF + WW
TOT = WOOFF + OW            # 57344

F32 = mybir.dt.float32
F16 = mybir.dt.float16

EXP = mybir.ActivationFunctionType.Exp
MULT = mybir.AluOpType.mult


def _emit_const(ctx: ExitStack, tc: tile.TileContext, blob):
    """Weights + persistent activation buffers: allocated/loaded once per
    NEFF; repeated executions of the body reuse resident weights."""
    nc = tc.nc
    const = ctx.enter_context(tc.tile_pool(name="const", bufs=1))
    sb = ctx.enter_context(tc.tile_pool(name="sb", bufs=1))
    # wk first: the k-projection starts the pipeline
    wk_sb = const.tile([P, WW], F16)
    nc.sync.dma_start(wk_sb[:], blob[:, WKOFF:WKOFF + WW])
    wq_sb = const.tile([P, WW], F16)
    nc.sync.dma_start(wq_sb[:], blob[:, WQOFF:WQOFF + WW])
    wv_sb = const.tile([P, WW], F16)
    nc.sync.dma_start(wv_sb[:], blob[:, WVOFF:WVOFF + WW])
    wo_sb = const.tile([P, OW], F16)
    nc.sync.dma_start(wo_sb[:], blob[:, WOOFF:WOOFF + OW])
    q_sb = [sb.tile([P, S], F16, name=f"q_sb{i}") for i in range(2)]
    k_sb = [sb.tile([P, S], F16, name=f"k_sb{i}") for i in range(2)]
    v_sb = [sb.tile([P, 2, 192], F16, name=f"v_sb{i}") for i in range(NKT)]
    ctx_sb = [sb.tile([P, S], F16, name=f"ctx_sb{i}") for i in range(2)]
    return (wk_sb, wq_sb, wv_sb, wo_sb, q_sb, k_sb, v_sb, ctx_sb)


def _emit(ctx: ExitStack, tc: tile.TileContext, hands, blob, outT):
    nc = tc.nc
    wk_sb, wq_sb, wv_sb, wo_sb, q_sb, k_sb, v_sb, ctx_sb = hands

    xtp = ctx.enter_context(tc.tile_pool(name="xtp", bufs=2))
    keepp = ctx.enter_context(tc.tile_pool(name="keepp", bufs=2))
    wp = ctx.enter_context(tc.tile_pool(name="wp", bufs=6))
    stg = ctx.enter_context(tc.tile_pool(name="stg", bufs=3))
    ps = ctx.enter_context(tc.tile_pool(name="ps", bufs=1, space="PSUM"))

    # ---- x chunks + first keep prefetch on the idle gpsimd queue ----
    xts = []
    for sc in range(NQC):
        xt = xtp.tile([P, XW], F16, tag="xt", name=f"xt_{sc}", bufs=NQC)
        nc.gpsimd.dma_start(xt[:], blob[:, XOFF + sc * XW:XOFF + (sc + 1) * XW])
        xts.append(xt)

    def keep_dma(qc):
        kt_ = keepp.tile([P, KW], F16, tag="keep", name=f"keep_{qc}")
        nc.gpsimd.dma_start(
            kt_[:], blob[:, KOFF + qc * KW:KOFF + (qc + 1) * KW])
        return kt_

    keeps = {0: keep_dma(0)}

    def proj_qk(w_sb, dst, pair, sc):
        # dst[pair][:, sc*SC:(sc+1)*SC] = (W x)[pair-feats, sc-chunk]
        mm = ps.tile([P, SC], F32, tag=("ctxY", "ctxX")[(pair + sc) % 2],
                     bufs=2, name=f"pj_{pair}_{sc}")
        for dt in range(NDT):
            nc.tensor.matmul(
                mm[:],
                w_sb[:, dt * FPC + pair * P:dt * FPC + (pair + 1) * P],
                xts[sc][:, dt * SC:(dt + 1) * SC],
                start=(dt == 0), stop=(dt == NDT - 1),
            )
        nc.vector.tensor_copy(dst[pair][:, sc * SC:(sc + 1) * SC], mm[:])

    def proj_v(kt):
        # v_sb[kt] rows = 128 s-positions of tile kt, cols = [va|ones|vb]
        sc, ssub = kt // (SC // P), kt % (SC // P)
        vm = ps.tile([P, FPC], F32, tag=("ctxY", "ctxX")[kt % 2], bufs=2,
                     name=f"v_{kt}")
        for dt in range(NDT):
            nc.tensor.matmul(
                vm[:],
                xts[sc][:, dt * SC + ssub * P:dt * SC + (ssub + 1) * P],
                wv_sb[:, dt * FPC:(dt + 1) * FPC],
                start=(dt == 0), stop=(dt == NDT - 1),
            )
        for pr in range(2):
            nc.vector.tensor_copy(v_sb[kt][:, pr, 0:DH],
                                  vm[:, pr * P:pr * P + DH])
            nc.vector.tensor_copy(v_sb[kt][:, pr, 2 * DH:3 * DH],
                                  vm[:, pr * P + DH:(pr + 1) * P])
        nc.gpsimd.memset(v_sb[kt][:, :, DH:2 * DH], 1.0)

    # fill: k fully (both pairs), then q for qc=0, then v tiles
    for pair in range(2):
        for sc in range(NQC):
            proj_qk(wk_sb, k_sb, pair, sc)
    for pair in range(2):
        proj_qk(wq_sb, q_sb, pair, 0)
    for kt in range(NKT):
        proj_v(kt)

    def om_items(qc):
        # output projection for qc as deferred items, drained into the next
        # qc's kt loop so ACT's exp stream is not interrupted at the
        # boundary. om tiles ride the "sc" PSUM rotation.
        qsl = slice(qc * SC, (qc + 1) * SC)
        st = stg.tile([P, NFT, SC], F16, tag="stage", bufs=2, name=f"st_{qc}")
        items = []
        for fth in range(NFT // 2):
            def f(fth=fth, qc=qc, st=st, qsl=qsl):
                # two ft chunks per 2-bank om span: half the sc-rotation
                # insertions, one wide eviction
                om = ps.tile([P, 2 * SC], F32, tag="sc", bufs=2,
                             name=f"o_{qc}_{fth}")
                for half in range(2):
                    ft = 2 * fth + half
                    osl = slice(half * SC, (half + 1) * SC)
                    for ph in range(FPC // P):
                        nc.tensor.matmul(
                            om[:, osl],
                            wo_sb[:, ph * D + ft * P:ph * D + (ft + 1) * P],
                            ctx_sb[ph][:, qsl],
                            start=(ph == 0), stop=(ph == FPC // P - 1),
                        )
                om3 = om[:].rearrange("p (f q) -> p f q", f=2)
                if fth % 2 == 0:
                    nc.scalar.copy(st[:, 2 * fth:2 * fth + 2, :], om3)
                else:
                    nc.vector.tensor_copy(st[:, 2 * fth:2 * fth + 2, :], om3)
            items.append(f)
        # output DMA split across two queues: halves the tail on the last qc
        items.append(lambda qc=qc, st=st: nc.sync.dma_start(
            outT[:, 0:NFT // 2, qc, :], st[:, 0:NFT // 2, :]))
        items.append(lambda qc=qc, st=st: nc.gpsimd.dma_start(
            outT[:, NFT // 2:NFT, qc, :], st[:, NFT // 2:NFT, :]))
        return items

    # ---- attention + per-qc output projection ----
    # `pending` holds deferred PE-side work (previous qc's output
    # projection, next qc's q-projection) drained one item per kt
    # iteration so the ACT exp stream never waits at qc boundaries.
    pending = []
    for qc in range(NQC):
        if qc + 1 < NQC:
            keeps[qc + 1] = keep_dma(qc + 1)
            def qp(pr, qc=qc):
                proj_qk(wq_sb, q_sb, pr, qc + 1)
            pending = pending + [lambda: qp(0), lambda: qp(1)]
        keep_sb = keeps.pop(qc)
        qsl = slice(qc * SC, (qc + 1) * SC)
        # both pair-streams interleaved per kt: doubles pipeline distance
        # between the PE->ACT->DVE->PE stages at the same PSUM budget
        ctx_y = [ps.tile([P, SC], F32, tag="ctxY", bufs=2,
                         name=f"ctxY_{qc}_{pair}") for pair in range(2)]
        ctx_x = [ps.tile([P, SC], F32, tag="ctxX", bufs=2,
                         name=f"ctxX_{qc}_{pair}") for pair in range(2)]
        # ctx accumulation lags scores/exp/mask by one kt: PE executes
        # in-order, so an un-lagged ctx matmul would bubble the PE queue
        # waiting on the DVE mask of its own kt.
        lagged = []

        def flush_ctx():
            pair, kt, w = lagged.pop(0)
            vt = v_sb[kt]
            first, last = kt == 0, kt == NKT - 1
            nc.tensor.matmul(
                ctx_y[pair][:], vt[:, pair, 0:2 * DH], w[:, 0:SC],
                start=first, stop=last,
            )
            nc.tensor.matmul(
                ctx_x[pair][:], vt[:, pair, DH:3 * DH], w[:, SC:2 * SC],
                start=first, stop=last,
            )

        for kt in range(NKT):
            for _ in range(2):
                if pending:
                    pending.pop(0)()
            ksl = slice(kt * P, (kt + 1) * P)
            for pair in range(2):
                # both heads' score tiles side by side in a 2-bank span
                scb = ps.tile([P, 2 * SC], F32, tag="sc", bufs=2,
                              name=f"scb_{qc}_{pair}_{kt}")
                nc.tensor.matmul(
                    scb[:, 0:SC],
                    k_sb[pair][0:DH, ksl],
                    q_sb[pair][0:DH, qsl],
                    start=True, stop=True,
                )
                nc.tensor.matmul(
                    scb[:, SC:2 * SC],
                    k_sb[pair][DH:P, ksl],
                    q_sb[pair][DH:P, qsl],
                    start=True, stop=True,
                    tile_position=(64, 0),
                )
                w = wp.tile([P, 2 * SC], F16, tag="w", name=f"w_{qc}_{pair}_{kt}")
                nc.scalar.activation(w[:], scb[:], EXP)
                w3 = w[:].rearrange("p (h q) -> p h q", h=2)
                kb = keep_sb[:, kt * SC:(kt + 1) * SC][:, None, :] \
                    .to_broadcast((P, 2, SC))
                nc.vector.tensor_tensor(w3, w3, kb, MULT)
                lagged.append((pair, kt, w))
                while len(lagged) > 2:
                    flush_ctx()
        while lagged:
            flush_ctx()
        for pair in range(2):
            # softmax normalization: denom_a sits (replicated over 64
            # partitions) on ctx_y[64:128], denom_b on ctx_x[0:64]. DVE
            # tensor_tensor allows a shifted-base SBUF operand when the
            # other input is PSUM, so normalize directly: pure DVE, no PE
            # broadcast matmuls, no PSUM rotation holds.
            cy, cx = ctx_y[pair], ctx_x[pair]
            recip = stg.tile([P, SC], F32, tag="recip", name=f"recip_{qc}_{pair}")
            nc.vector.reciprocal(recip[0:DH, :], cx[0:DH, :])
            nc.vector.reciprocal(recip[DH:P, :], cy[DH:P, :])
            nc.vector.tensor_tensor(
                ctx_sb[pair][0:DH, qsl], cy[0:DH, :], recip[DH:P, :], MULT)
            nc.vector.tensor_tensor(
                ctx_sb[pair][DH:P, qsl], cx[DH:P, :], recip[0:DH, :], MULT)
        for it in pending:  # anything not yet drained (shouldn't be much)
            it()
        if qc + 1 < NQC:
            pending = om_items(qc)
        else:
            pending = []
            for it in om_items(qc):
                it()


def build(repeat=1):
    nc = bacc.Bacc("TRN2", target_bir_lowering=False, debug=False,
                   num_devices=NCORES)
    blob = nc.dram_tensor("blob", [P, TOT], F16, kind="ExternalInput").ap()
    outT = nc.dram_tensor("outT", [P, NFT, NQC, SC], F16,
                          kind="ExternalOutput").ap()
    with tile.TileContext(nc) as tc, ExitStack() as cctx:
        hands = _emit_const(cctx, tc, blob)
        for _ in range(repeat):
            with ExitStack() as ctx:
                _emit(ctx, tc, hands, blob, outT)
    nc.compile()
    return nc


def make_in_maps(query, mask, Wq, Wk, Wv, Wo):
    scale = 1.0 / math.sqrt(DH)
    in_maps = []
    for b in range(B):
        # x section: [P, NQC, NDT, SC]; elem (p, sc, dt, s) = x[sc*SC+s, dt*P+p]
        xt = query[b].T.reshape(NDT, P, NQC, SC).transpose(1, 2, 0, 3)
        xsec = np.ascontiguousarray(xt, dtype=np.float16).reshape(P, NQC * XW)
        # keep section: [P, NQC, NKT, SC]; (p, qc, kt, q) = keep[kt*P+p, qc*SC+q]
        kp = (~mask[b]).T.reshape(NKT, P, NQC, SC).transpose(1, 2, 0, 3)
        ksec = np.ascontiguousarray(kp, dtype=np.float16).reshape(P, NQC * KW)
        for g in range(GROUPS):
            f0 = g * FPC

            def pack_w(wT):  # [D, FPC] -> [P, NDT*FPC] ([p, dt*FPC+f])
                return np.ascontiguousarray(
                    wT.reshape(NDT, P, FPC).transpose(1, 0, 2),
                    dtype=np.float16).reshape(P, WW)

            wosec = np.ascontiguousarray(
                Wo[:, f0:f0 + FPC].T.reshape(FPC // P, P, D).transpose(1, 0, 2),
                dtype=np.float16).reshape(P, OW)
            blob = np.concatenate([
                xsec, ksec,
                pack_w((Wq[f0:f0 + FPC, :] * scale).T),
                pack_w(Wk[f0:f0 + FPC, :].T),
                pack_w(Wv[f0:f0 + FPC, :].T),
                wosec,
            ], axis=1)
            assert blob.shape == (P, TOT) and blob.dtype == np.float16
            in_maps.append({"blob": blob})
    return in_maps


_NC_CACHE = {}


def _get_nc():
    if "nc" not in _NC_CACHE:
        _NC_CACHE["nc"] = build()
    return _NC_CACHE["nc"]


def gather(results, bo):
    out = np.empty((B, S, D), dtype=np.float32)
    for b in range(B):
        acc = results[b * GROUPS]["outT"].astype(np.float32)
        for g in range(1, GROUPS):
            acc = acc + results[b * GROUPS + g]["outT"]
        # [P, NFT, NQC, SC] -> [D, S]: feature f = ft*P + p, pos s = qc*SC + q
        full = acc.transpose(1, 0, 2, 3).reshape(D, S)
        out[b] = full.T + bo.astype(np.float32)
    return out


def kernel(query, mask, Wq, Wk, Wv, Wo, bo, **kwargs):
    nc = _get_nc()
    in_maps = make_in_maps(np.asarray(query), np.asarray(mask), np.asarray(Wq),
                           np.asarray(Wk), np.asarray(Wv), np.asarray(Wo))
    res = run_bass_kernel_spmd(nc, in_maps, list(range(NCORES)))
    return gather(res.results, np.asarray(bo))


# revision 33
# speedup vs baseline: 1.0855x; 1.0855x over previous
"""Multi-headed attention TRN2 Bass kernel (v2).

Problem: B=2, S=2048, D=1024, H=16 heads (dh=64), fp32 in/out, bool mask.

Sharding (8 cores): data-parallel over B (2) x tensor-parallel over heads
(4 heads / 256 features per core). Each core computes its head-group's
q/k/v projections, masked softmax attention, and a partial output
projection (Wo columns for its heads). Host sums the 4 partials per batch
element (the TP all-reduce) and adds the bias.

v2 changes vs v1:
  - ONE input tensor ("blob") and ONE output per core. The per-call
    dispatch overhead of the axon/PJRT path scales ~60us per sharded
    buffer per exec; 7 buffers -> 2 buffers roughly halves the measured
    wall time before any device-side change.
  - fp16 everywhere (x, weights, keep-mask, output partials): half the
    DMA bytes, DVE 2x/4x fast modes, 1 cycle/row matmuls.
  - exp on ACT over [128, 1024] PSUM spans (two banks / both heads of a
    pair per call) to amortize the ~352-cycle ACT fixed overhead.
  - all mask multiplies on DVE (fp16 4x mode); prefetch DMAs issued from
    the idle gpsimd queue; emission order overlaps k/q0 projection fill
    with attention and runs the output projection per-qc instead of at
    the end.

Per-core layout (all fp16, feature/key-position major a.k.a. transposed):
  blob cols: [x: NQC*NDT*SC][keep: NQC*NKT*SC][wq|wk|wv: NDT*FPC][wo: 2*D]
  out: [P, NFT=8, NQC, SC] partial output (summed across 4 cores on host).

No max-subtraction in softmax: scores ~N(0,1), exp in fp32->fp16 is
plenty within the 2e-2 gate (measured ~5e-4 end to end).
"""

import math
from contextlib import ExitStack

import numpy as np

import concourse.mybir as mybir
import concourse.tile as tile
from concourse import bacc
from concourse.bass_utils import run_bass_kernel_spmd

B, S, D, H = 2, 2048, 1024, 16
DH = D // H                 # 64
NCORES = 8
GROUPS = NCORES // B        # 4 head-groups per batch element
FPC = D // GROUPS           # 256 features (4 heads) per core
P = 128
SC = 512                    # q/s chunk (free dim of most matmuls)
NQC = S // SC               # 4
NKT = S // P                # 16 k-position tiles
NDT = D // P                # 8 contraction tiles over D
NFT = D // P                # 8 output-feature tiles

# blob column offsets (fp16 elements per partition)
XW = NDT * SC               # 4096 cols per x chunk
KW = NKT * SC               # 8192 cols per keep chunk
WW = NDT * FPC              # 2048 cols per qkv weight
OW = (FPC // P) * D         # 2048 cols for wo
XOFF = 0
KOFF = XOFF + NQC * XW      # 16384
WQOFF = KOFF + NQC * KW     # 49152
WKOFF = WQOFF + WW
WVOFF = WKOFF + WW
WOOFF = WVOFF + WW
TOT = WOOFF + OW            # 57344

F32 = mybir.dt.float32
F16 = mybir.dt.float16

EXP = mybir.ActivationFunctionType.Exp
MULT = mybir.AluOpType.mult


def _emit_const(ctx: ExitStack, tc: tile.TileContext, blob):
    """Weights + persistent activation buffers: allocated/loaded once per
    NEFF; repeated executions of the body reuse resident weights."""
    nc = tc.nc
    const = ctx.enter_context(tc.tile_pool(name="const", bufs=1))
    sb = ctx.enter_context(tc.tile_pool(name="sb", bufs=1))
    # wk first: the k-projection starts the pipeline
    wk_sb = const.tile([P, WW], F16)
    nc.sync.dma_start(wk_sb[:], blob[:, WKOFF:WKOFF + WW])
    wq_sb = const.tile([P, WW], F16)
    nc.sync.dma_start(wq_sb[:], blob[:, WQOFF:WQOFF + WW])
    wv_sb = const.tile([P, WW], F16)
    nc.sync.dma_start(wv_sb[:], blob[:, WVOFF:WVOFF + WW])
    wo_sb = const.tile([P, OW], F16)
    nc.sync.dma_start(wo_sb[:], blob[:, WOOFF:WOOFF + OW])
    q_sb = [sb.tile([P, S], F16, name=f"q_sb{i}") for i in range(2)]
    k_sb = [sb.tile([P, S], F16, name=f"k_sb{i}") for i in range(2)]
    v_sb = [sb.tile([P, 2, 192], F16, name=f"v_sb{i}") for i in range(NKT)]
    ctx_sb = [sb.tile([P, S], F16, name=f"ctx_sb{i}") for i in range(2)]
    return (wk_sb, wq_sb, wv_sb, wo_sb, q_sb, k_sb, v_sb, ctx_sb)


def _emit_pools(ctx: ExitStack, tc: tile.TileContext):
    """Transient pools, hoisted out of the repeat loop: tag rotation then
    spans executions, so one exec's fill overlaps the previous exec's tail
    instead of serializing on SBUF address reuse."""
    return dict(
        xtp=ctx.enter_context(tc.tile_pool(name="xtp", bufs=2)),
        keepp=ctx.enter_context(tc.tile_pool(name="keepp", bufs=3)),
        wp=ctx.enter_context(tc.tile_pool(name="wp", bufs=6)),
        stg=ctx.enter_context(tc.tile_pool(name="stg", bufs=3)),
        ps=ctx.enter_context(tc.tile_pool(name="ps", bufs=1, space="PSUM")),
    )


_EMIT_SEQ = [0]


def _emit(pools, tc: tile.TileContext, hands, blob, outT):
    nc = tc.nc
    wk_sb, wq_sb, wv_sb, wo_sb, q_sb, k_sb, v_sb, ctx_sb = hands
    xtp, keepp, wp, stg, ps = (pools["xtp"], pools["keepp"], pools["wp"],
                               pools["stg"], pools["ps"])
    _EMIT_SEQ[0] += 1
    rep = _EMIT_SEQ[0]

    # ---- x chunks + first keep prefetch on the idle gpsimd queue ----
    xts = []
    for sc in range(NQC):
        xt = xtp.tile([P, XW], F16, tag="xt", name=f"xt_{rep}_{sc}",
                      bufs=2 * NQC)
        nc.gpsimd.dma_start(xt[:], blob[:, XOFF + sc * XW:XOFF + (sc + 1) * XW])
        xts.append(xt)

    def keep_dma(qc):
        kt_ = keepp.tile([P, KW], F16, tag="keep", name=f"keep_{qc}")
        nc.gpsimd.dma_start(
            kt_[:], blob[:, KOFF + qc * KW:KOFF + (qc + 1) * KW])
        return kt_

    keeps = {0: keep_dma(0)}

    def proj_qk(w_sb, dst, pair, sc):
        # dst[pair][:, sc*SC:(sc+1)*SC] = (W x)[pair-feats, sc-chunk]
        mm = ps.tile([P, SC], F32, tag=("ctxY", "ctxX")[(pair + sc) % 2],
                     bufs=2, name=f"pj_{pair}_{sc}")
        for dt in range(NDT):
            nc.tensor.matmul(
                mm[:],
                w_sb[:, dt * FPC + pair * P:dt * FPC + (pair + 1) * P],
                xts[sc][:, dt * SC:(dt + 1) * SC],
                start=(dt == 0), stop=(dt == NDT - 1),
            )
        nc.vector.tensor_copy(dst[pair][:, sc * SC:(sc + 1) * SC], mm[:])

    def proj_v(kt):
        # v_sb[kt] rows = 128 s-positions of tile kt, cols = [va|ones|vb]
        sc, ssub = kt // (SC // P), kt % (SC // P)
        vm = ps.tile([P, FPC], F32, tag=("ctxY", "ctxX")[kt % 2], bufs=2,
                     name=f"v_{kt}")
        for dt in range(NDT):
            nc.tensor.matmul(
                vm[:],
                xts[sc][:, dt * SC + ssub * P:dt * SC + (ssub + 1) * P],
                wv_sb[:, dt * FPC:(dt + 1) * FPC],
                start=(dt == 0), stop=(dt == NDT - 1),
            )
        for pr in range(2):
            nc.vector.tensor_copy(v_sb[kt][:, pr, 0:DH],
                                  vm[:, pr * P:pr * P + DH])
            nc.vector.tensor_copy(v_sb[kt][:, pr, 2 * DH:3 * DH],
                                  vm[:, pr * P + DH:(pr + 1) * P])
        nc.gpsimd.memset(v_sb[kt][:, :, DH:2 * DH], 1.0)

    # fill: k fully (both pairs), then q for qc=0, then v tiles
    for pair in range(2):
        for sc in range(NQC):
            proj_qk(wk_sb, k_sb, pair, sc)
    for pair in range(2):
        proj_qk(wq_sb, q_sb, pair, 0)
    for kt in range(NKT):
        proj_v(kt)

    def om_items(qc):
        # output projection for qc as deferred items, drained into the next
        # qc's kt loop so ACT's exp stream is not interrupted at the
        # boundary. om tiles ride the "sc" PSUM rotation.
        qsl = slice(qc * SC, (qc + 1) * SC)
        st = stg.tile([P, NFT, SC], F16, tag="stage", bufs=2, name=f"st_{qc}")
        items = []
        for fth in range(NFT // 2):
            def f(fth=fth, qc=qc, st=st, qsl=qsl):
                # two ft chunks per 2-bank om span: half the sc-rotation
                # insertions, one wide eviction
                om = ps.tile([P, 2 * SC], F32, tag="sc", bufs=2,
                             name=f"o_{qc}_{fth}")
                for half in range(2):
                    ft = 2 * fth + half
                    osl = slice(half * SC, (half + 1) * SC)
                    for ph in range(FPC // P):
                        nc.tensor.matmul(
                            om[:, osl],
                            wo_sb[:, ph * D + ft * P:ph * D + (ft + 1) * P],
                            ctx_sb[ph][:, qsl],
                            start=(ph == 0), stop=(ph == FPC // P - 1),
                        )
                om3 = om[:].rearrange("p (f q) -> p f q", f=2)
                nc.vector.tensor_copy(st[:, 2 * fth:2 * fth + 2, :], om3)
            items.append(f)
        # output DMA split across two queues: halves the tail on the last qc
        items.append(lambda qc=qc, st=st: nc.sync.dma_start(
            outT[:, 0:NFT // 2, qc, :], st[:, 0:NFT // 2, :]))
        items.append(lambda qc=qc, st=st: nc.gpsimd.dma_start(
            outT[:, NFT // 2:NFT, qc, :], st[:, NFT // 2:NFT, :]))
        return items

    # ---- attention + per-qc output projection ----
    # `pending` holds deferred PE-side work (previous qc's output
    # projection, next qc's q-projection) drained one item per kt
    # iteration so the ACT exp stream never waits at qc boundaries.
    pending = []
    for qc in range(NQC):
        if qc + 1 < NQC:
            keeps[qc + 1] = keep_dma(qc + 1)
            def qp(pr, qc=qc):
                proj_qk(wq_sb, q_sb, pr, qc + 1)
            pending = pending + [lambda: qp(0), lambda: qp(1)]
        keep_sb = keeps.pop(qc)
        qsl = slice(qc * SC, (qc + 1) * SC)
        # both pair-streams interleaved per kt: doubles pipeline distance
        # between the PE->ACT->DVE->PE stages at the same PSUM budget
        ctx_y = [ps.tile([P, SC], F32, tag="ctxY", bufs=2,
                         name=f"ctxY_{qc}_{pair}") for pair in range(2)]
        ctx_x = [ps.tile([P, SC], F32, tag="ctxX", bufs=2,
                         name=f"ctxX_{qc}_{pair}") for pair in range(2)]
        # ctx accumulation lags scores/exp/mask by one kt: PE executes
        # in-order, so an un-lagged ctx matmul would bubble the PE queue
        # waiting on the DVE mask of its own kt.
        lagged = []

        def flush_ctx():
            pair, kt, w = lagged.pop(0)
            vt = v_sb[kt]
            first, last = kt == 0, kt == NKT - 1
            nc.tensor.matmul(
                ctx_y[pair][:], vt[:, pair, 0:2 * DH], w[:, 0:SC],
                start=first, stop=last,
            )
            nc.tensor.matmul(
                ctx_x[pair][:], vt[:, pair, DH:3 * DH], w[:, SC:2 * SC],
                start=first, stop=last,
            )

        for kt in range(NKT):
            for _ in range(2):
                if pending:
                    pending.pop(0)()
            ksl = slice(kt * P, (kt + 1) * P)
            for pair in range(2):
                # both heads' score tiles side by side in a 2-bank span
                scb = ps.tile([P, 2 * SC], F32, tag="sc", bufs=2,
                              name=f"scb_{qc}_{pair}_{kt}")
                nc.tensor.matmul(
                    scb[:, 0:SC],
                    k_sb[pair][0:DH, ksl],
                    q_sb[pair][0:DH, qsl],
                    start=True, stop=True,
                )
                nc.tensor.matmul(
                    scb[:, SC:2 * SC],
                    k_sb[pair][DH:P, ksl],
                    q_sb[pair][DH:P, qsl],
                    start=True, stop=True,
                    tile_position=(64, 0),
                )
                w = wp.tile([P, 2 * SC], F16, tag="w", name=f"w_{qc}_{pair}_{kt}")
                nc.scalar.activation(w[:], scb[:], EXP)
                w3 = w[:].rearrange("p (h q) -> p h q", h=2)
                kb = keep_sb[:, kt * SC:(kt + 1) * SC][:, None, :] \
                    .to_broadcast((P, 2, SC))
                nc.vector.tensor_tensor(w3, w3, kb, MULT)
                lagged.append((pair, kt, w))
                while len(lagged) > 2:
                    flush_ctx()
        while lagged:
            flush_ctx()
        for pair in range(2):
            # softmax normalization: denom_a sits (replicated over 64
            # partitions) on ctx_y[64:128], denom_b on ctx_x[0:64]. DVE
            # tensor_tensor allows a shifted-base SBUF operand when the
            # other input is PSUM, so normalize directly: pure DVE, no PE
            # broadcast matmuls, no PSUM rotation holds.
            cy, cx = ctx_y[pair], ctx_x[pair]
            recip = stg.tile([P, SC], F32, tag="recip", name=f"recip_{qc}_{pair}")
            nc.vector.reciprocal(recip[0:DH, :], cx[0:DH, :])
            nc.vector.reciprocal(recip[DH:P, :], cy[DH:P, :])
            nc.vector.tensor_tensor(
                ctx_sb[pair][0:DH, qsl], cy[0:DH, :], recip[DH:P, :], MULT)
            nc.vector.tensor_tensor(
                ctx_sb[pair][DH:P, qsl], cx[DH:P, :], recip[0:DH, :], MULT)
        for it in pending:  # anything not yet drained (shouldn't be much)
            it()
        if qc + 1 < NQC:
            pending = om_items(qc)
        else:
            pending = []
            for it in om_items(qc):
                it()


def build(repeat=1):
    nc = bacc.Bacc("TRN2", target_bir_lowering=False, debug=False,
                   num_devices=NCORES)
    blob = nc.dram_tensor("blob", [P, TOT], F16, kind="ExternalInput").ap()
    outT = nc.dram_tensor("outT", [P, NFT, NQC, SC], F16,
                          kind="ExternalOutput").ap()
    with tile.TileContext(nc) as tc, ExitStack() as cctx:
        hands = _emit_const(cctx, tc, blob)
        pools = _emit_pools(cctx, tc)
        for _ in range(repeat):
            _emit(pools, tc, hands, blob, outT)
    nc.compile()
    return nc


def make_in_maps(query, mask, Wq, Wk, Wv, Wo):
    scale = 1.0 / math.sqrt(DH)
    in_maps = []
    for b in range(B):
        # x section: [P, NQC, NDT, SC]; elem (p, sc, dt, s) = x[sc*SC+s, dt*P+p]
        xt = query[b].T.reshape(NDT, P, NQC, SC).transpose(1, 2, 0, 3)
        xsec = np.ascontiguousarray(xt, dtype=np.float16).reshape(P, NQC * XW)
        # keep section: [P, NQC, NKT, SC]; (p, qc, kt, q) = keep[kt*P+p, qc*SC+q]
        kp = (~mask[b]).T.reshape(NKT, P, NQC, SC).transpose(1, 2, 0, 3)
        ksec = np.ascontiguousarray(kp, dtype=np.float16).reshape(P, NQC * KW)
        for g in range(GROUPS):
            f0 = g * FPC

            def pack_w(wT):  # [D, FPC] -> [P, NDT*FPC] ([p, dt*FPC+f])
                return np.ascontiguousarray(
                    wT.reshape(NDT, P, FPC).transpose(1, 0, 2),
                    dtype=np.float16).reshape(P, WW)

            wosec = np.ascontiguousarray(
                Wo[:, f0:f0 + FPC].T.reshape(FPC // P, P, D).transpose(1, 0, 2),
                dtype=np.float16).reshape(P, OW)
            blob = np.concatenate([
                xsec, ksec,
                pack_w((Wq[f0:f0 + FPC, :] * scale).T),
                pack_w(Wk[f0:f0 + FPC, :].T),
                pack_w(Wv[f0:f0 + FPC, :].T),
                wosec,
            ], axis=1)
            assert blob.shape == (P, TOT) and blob.dtype == np.float16
            in_maps.append({"blob": blob})
    return in_maps


_NC_CACHE = {}


def _get_nc():
    if "nc" not in _NC_CACHE:
        _NC_CACHE["nc"] = build()
    return _NC_CACHE["nc"]


def gather(results, bo):
    out = np.empty((B, S, D), dtype=np.float32)
    for b in range(B):
        acc = results[b * GROUPS]["outT"].astype(np.float32)
        for g in range(1, GROUPS):
            acc = acc + results[b * GROUPS + g]["outT"]
        # [P, NFT, NQC, SC] -> [D, S]: feature f = ft*P + p, pos s = qc*SC + q
        full = acc.transpose(1, 0, 2, 3).reshape(D, S)
        out[b] = full.T + bo.astype(np.float32)
    return out


def kernel(query, mask, Wq, Wk, Wv, Wo, bo, **kwargs):
    nc = _get_nc()
    in_maps = make_in_maps(np.asarray(query), np.asarray(mask), np.asarray(Wq),
                           np.asarray(Wk), np.asarray(Wv), np.asarray(Wo))
    res = run_bass_kernel_spmd(nc, in_maps, list(range(NCORES)))
    return gather(res.results, np.asarray(bo))


# revision 35
# speedup vs baseline: 1.1217x; 1.0333x over previous
"""Multi-headed attention TRN2 Bass kernel (v2).

Problem: B=2, S=2048, D=1024, H=16 heads (dh=64), fp32 in/out, bool mask.

Sharding (8 cores): data-parallel over B (2) x tensor-parallel over heads
(4 heads / 256 features per core). Each core computes its head-group's
q/k/v projections, masked softmax attention, and a partial output
projection (Wo columns for its heads). Host sums the 4 partials per batch
element (the TP all-reduce) and adds the bias.

v2 changes vs v1:
  - ONE input tensor ("blob") and ONE output per core. The per-call
    dispatch overhead of the axon/PJRT path scales ~60us per sharded
    buffer per exec; 7 buffers -> 2 buffers roughly halves the measured
    wall time before any device-side change.
  - fp16 everywhere (x, weights, keep-mask, output partials): half the
    DMA bytes, DVE 2x/4x fast modes, 1 cycle/row matmuls.
  - exp on ACT over [128, 1024] PSUM spans (two banks / both heads of a
    pair per call) to amortize the ~352-cycle ACT fixed overhead.
  - all mask multiplies on DVE (fp16 4x mode); prefetch DMAs issued from
    the idle gpsimd queue; emission order overlaps k/q0 projection fill
    with attention and runs the output projection per-qc instead of at
    the end.

Per-core layout (all fp16, feature/key-position major a.k.a. transposed):
  blob cols: [x: NQC*NDT*SC][keep: NQC*NKT*SC][wq|wk|wv: NDT*FPC][wo: 2*D]
  out: [P, NFT=8, NQC, SC] partial output (summed across 4 cores on host).

No max-subtraction in softmax: scores ~N(0,1), exp in fp32->fp16 is
plenty within the 2e-2 gate (measured ~5e-4 end to end).
"""

import math
from contextlib import ExitStack

import numpy as np

import concourse.mybir as mybir
import concourse.tile as tile
from concourse import bacc
from concourse.bass_utils import run_bass_kernel_spmd

B, S, D, H = 2, 2048, 1024, 16
DH = D // H                 # 64
NCORES = 8
GROUPS = NCORES // B        # 4 head-groups per batch element
FPC = D // GROUPS           # 256 features (4 heads) per core
P = 128
SC = 512                    # q/s chunk (free dim of most matmuls)
NQC = S // SC               # 4
NKT = S // P                # 16 k-position tiles
NDT = D // P                # 8 contraction tiles over D
NFT = D // P                # 8 output-feature tiles

# blob column offsets (fp16 elements per partition)
XW = NDT * SC               # 4096 cols per x chunk
KW = NKT * SC               # 8192 cols per keep chunk
WW = NDT * FPC              # 2048 cols per qkv weight
OW = (FPC // P) * D         # 2048 cols for wo
XOFF = 0
KOFF = XOFF + NQC * XW      # 16384
WQOFF = KOFF + NQC * KW     # 49152
WKOFF = WQOFF + WW
WVOFF = WKOFF + WW
WOOFF = WVOFF + WW
TOT = WOOFF + OW            # 57344

F32 = mybir.dt.float32
F16 = mybir.dt.float16

EXP = mybir.ActivationFunctionType.Exp
MULT = mybir.AluOpType.mult


def _emit_const(ctx: ExitStack, tc: tile.TileContext, blob):
    """Weights + persistent activation buffers: allocated/loaded once per
    NEFF; repeated executions of the body reuse resident weights."""
    nc = tc.nc
    const = ctx.enter_context(tc.tile_pool(name="const", bufs=1))
    sb = ctx.enter_context(tc.tile_pool(name="sb", bufs=1))
    # wk first: the k-projection starts the pipeline
    wk_sb = const.tile([P, WW], F16)
    nc.sync.dma_start(wk_sb[:], blob[:, WKOFF:WKOFF + WW])
    wq_sb = const.tile([P, WW], F16)
    nc.sync.dma_start(wq_sb[:], blob[:, WQOFF:WQOFF + WW])
    wv_sb = const.tile([P, WW], F16)
    nc.sync.dma_start(wv_sb[:], blob[:, WVOFF:WVOFF + WW])
    wo_sb = const.tile([P, OW], F16)
    nc.sync.dma_start(wo_sb[:], blob[:, WOOFF:WOOFF + OW])
    q_sb = [sb.tile([P, S], F16, name=f"q_sb{i}") for i in range(2)]
    k_sb = [sb.tile([P, S], F16, name=f"k_sb{i}") for i in range(2)]
    v_sb = [sb.tile([P, 2, 192], F16, name=f"v_sb{i}") for i in range(NKT)]
    ctx_sb = [sb.tile([P, S], F16, name=f"ctx_sb{i}") for i in range(2)]
    for vt in v_sb:  # constant denominator ride-along columns
        nc.gpsimd.memset(vt[:, :, DH:2 * DH], 1.0)
    return (wk_sb, wq_sb, wv_sb, wo_sb, q_sb, k_sb, v_sb, ctx_sb)


def _emit_pools(ctx: ExitStack, tc: tile.TileContext):
    """Transient pools, hoisted out of the repeat loop: tag rotation then
    spans executions, so one exec's fill overlaps the previous exec's tail
    instead of serializing on SBUF address reuse."""
    return dict(
        xtp=ctx.enter_context(tc.tile_pool(name="xtp", bufs=2)),
        keepp=ctx.enter_context(tc.tile_pool(name="keepp", bufs=3)),
        wp=ctx.enter_context(tc.tile_pool(name="wp", bufs=6)),
        stg=ctx.enter_context(tc.tile_pool(name="stg", bufs=3)),
        ps=ctx.enter_context(tc.tile_pool(name="ps", bufs=1, space="PSUM")),
    )


_EMIT_SEQ = [0]


def _emit(pools, tc: tile.TileContext, hands, blob, outT):
    nc = tc.nc
    wk_sb, wq_sb, wv_sb, wo_sb, q_sb, k_sb, v_sb, ctx_sb = hands
    xtp, keepp, wp, stg, ps = (pools["xtp"], pools["keepp"], pools["wp"],
                               pools["stg"], pools["ps"])
    _EMIT_SEQ[0] += 1
    rep = _EMIT_SEQ[0]

    # ---- x chunks + first keep prefetch on the idle gpsimd queue ----
    xts = []
    for sc in range(NQC):
        xt = xtp.tile([P, XW], F16, tag="xt", name=f"xt_{rep}_{sc}",
                      bufs=2 * NQC)
        nc.gpsimd.dma_start(xt[:], blob[:, XOFF + sc * XW:XOFF + (sc + 1) * XW])
        xts.append(xt)

    def keep_dma(qc):
        kt_ = keepp.tile([P, KW], F16, tag="keep", name=f"keep_{qc}")
        nc.gpsimd.dma_start(
            kt_[:], blob[:, KOFF + qc * KW:KOFF + (qc + 1) * KW])
        return kt_

    keeps = {0: keep_dma(0)}

    def proj_qk(w_sb, dst, pair, sc):
        # dst[pair][:, sc*SC:(sc+1)*SC] = (W x)[pair-feats, sc-chunk]
        mm = ps.tile([P, SC], F32, tag=("ctxY", "ctxX")[(pair + sc) % 2],
                     bufs=2, name=f"pj_{pair}_{sc}")
        for dt in range(NDT):
            nc.tensor.matmul(
                mm[:],
                w_sb[:, dt * FPC + pair * P:dt * FPC + (pair + 1) * P],
                xts[sc][:, dt * SC:(dt + 1) * SC],
                start=(dt == 0), stop=(dt == NDT - 1),
            )
        nc.vector.tensor_copy(dst[pair][:, sc * SC:(sc + 1) * SC], mm[:])

    def proj_v(kt):
        # v_sb[kt] rows = 128 s-positions of tile kt, cols = [va|ones|vb]
        sc, ssub = kt // (SC // P), kt % (SC // P)
        vm = ps.tile([P, FPC], F32, tag=("ctxY", "ctxX")[kt % 2], bufs=2,
                     name=f"v_{kt}")
        for dt in range(NDT):
            nc.tensor.matmul(
                vm[:],
                xts[sc][:, dt * SC + ssub * P:dt * SC + (ssub + 1) * P],
                wv_sb[:, dt * FPC:(dt + 1) * FPC],
                start=(dt == 0), stop=(dt == NDT - 1),
            )
        for pr in range(2):
            nc.vector.tensor_copy(v_sb[kt][:, pr, 0:DH],
                                  vm[:, pr * P:pr * P + DH])
            nc.vector.tensor_copy(v_sb[kt][:, pr, 2 * DH:3 * DH],
                                  vm[:, pr * P + DH:(pr + 1) * P])
        # (ones ride-along regions are constant; memset once in _emit_const)

    # fill: k fully (both pairs), then q for qc=0, then v tiles
    for pair in range(2):
        for sc in range(NQC):
            proj_qk(wk_sb, k_sb, pair, sc)
    for pair in range(2):
        proj_qk(wq_sb, q_sb, pair, 0)
    for kt in range(NKT):
        proj_v(kt)

    def om_items(qc):
        # output projection for qc as deferred items, drained into the next
        # qc's kt loop so ACT's exp stream is not interrupted at the
        # boundary. om tiles ride the "sc" PSUM rotation.
        qsl = slice(qc * SC, (qc + 1) * SC)
        st = stg.tile([P, NFT, SC], F16, tag="stage", bufs=2, name=f"st_{qc}")
        items = []
        for fth in range(NFT // 2):
            def f(fth=fth, qc=qc, st=st, qsl=qsl):
                # two ft chunks per 2-bank om span: half the sc-rotation
                # insertions, one wide eviction
                om = ps.tile([P, 2 * SC], F32, tag="sc", bufs=2,
                             name=f"o_{qc}_{fth}")
                for half in range(2):
                    ft = 2 * fth + half
                    osl = slice(half * SC, (half + 1) * SC)
                    for ph in range(FPC // P):
                        nc.tensor.matmul(
                            om[:, osl],
                            wo_sb[:, ph * D + ft * P:ph * D + (ft + 1) * P],
                            ctx_sb[ph][:, qsl],
                            start=(ph == 0), stop=(ph == FPC // P - 1),
                        )
                om3 = om[:].rearrange("p (f q) -> p f q", f=2)
                nc.vector.tensor_copy(st[:, 2 * fth:2 * fth + 2, :], om3)
            items.append(f)
        # output DMA split across two queues: halves the tail on the last qc
        items.append(lambda qc=qc, st=st: nc.sync.dma_start(
            outT[:, 0:NFT // 2, qc, :], st[:, 0:NFT // 2, :]))
        items.append(lambda qc=qc, st=st: nc.gpsimd.dma_start(
            outT[:, NFT // 2:NFT, qc, :], st[:, NFT // 2:NFT, :]))
        return items

    # ---- attention + per-qc output projection ----
    # `pending` holds deferred PE-side work (previous qc's output
    # projection, next qc's q-projection) drained one item per kt
    # iteration so the ACT exp stream never waits at qc boundaries.
    pending = []
    for qc in range(NQC):
        if qc + 1 < NQC:
            keeps[qc + 1] = keep_dma(qc + 1)
            def qp(pr, qc=qc):
                proj_qk(wq_sb, q_sb, pr, qc + 1)
            pending = pending + [lambda: qp(0), lambda: qp(1)]
        keep_sb = keeps.pop(qc)
        qsl = slice(qc * SC, (qc + 1) * SC)
        # both pair-streams interleaved per kt: doubles pipeline distance
        # between the PE->ACT->DVE->PE stages at the same PSUM budget
        ctx_y = [ps.tile([P, SC], F32, tag="ctxY", bufs=2,
                         name=f"ctxY_{qc}_{pair}") for pair in range(2)]
        ctx_x = [ps.tile([P, SC], F32, tag="ctxX", bufs=2,
                         name=f"ctxX_{qc}_{pair}") for pair in range(2)]
        # ctx accumulation lags scores/exp/mask by one kt: PE executes
        # in-order, so an un-lagged ctx matmul would bubble the PE queue
        # waiting on the DVE mask of its own kt.
        lagged = []

        def flush_ctx():
            pair, kt, w = lagged.pop(0)
            vt = v_sb[kt]
            first, last = kt == 0, kt == NKT - 1
            nc.tensor.matmul(
                ctx_y[pair][:], vt[:, pair, 0:2 * DH], w[:, 0:SC],
                start=first, stop=last,
            )
            nc.tensor.matmul(
                ctx_x[pair][:], vt[:, pair, DH:3 * DH], w[:, SC:2 * SC],
                start=first, stop=last,
            )

        for kt in range(NKT):
            for _ in range(2):
                if pending:
                    pending.pop(0)()
            ksl = slice(kt * P, (kt + 1) * P)
            for pair in range(2):
                # both heads' score tiles side by side in a 2-bank span
                scb = ps.tile([P, 2 * SC], F32, tag="sc", bufs=2,
                              name=f"scb_{qc}_{pair}_{kt}")
                nc.tensor.matmul(
                    scb[:, 0:SC],
                    k_sb[pair][0:DH, ksl],
                    q_sb[pair][0:DH, qsl],
                    start=True, stop=True,
                )
                nc.tensor.matmul(
                    scb[:, SC:2 * SC],
                    k_sb[pair][DH:P, ksl],
                    q_sb[pair][DH:P, qsl],
                    start=True, stop=True,
                    tile_position=(64, 0),
                )
                w = wp.tile([P, 2 * SC], F16, tag="w", name=f"w_{qc}_{pair}_{kt}")
                nc.scalar.activation(w[:], scb[:], EXP)
                w3 = w[:].rearrange("p (h q) -> p h q", h=2)
                kb = keep_sb[:, kt * SC:(kt + 1) * SC][:, None, :] \
                    .to_broadcast((P, 2, SC))
                nc.vector.tensor_tensor(w3, w3, kb, MULT)
                lagged.append((pair, kt, w))
                while len(lagged) > 2:
                    flush_ctx()
        while lagged:
            flush_ctx()
        for pair in range(2):
            # softmax normalization: denom_a sits (replicated over 64
            # partitions) on ctx_y[64:128], denom_b on ctx_x[0:64]. DVE
            # tensor_tensor allows a shifted-base SBUF operand when the
            # other input is PSUM, so normalize directly: pure DVE, no PE
            # broadcast matmuls, no PSUM rotation holds.
            cy, cx = ctx_y[pair], ctx_x[pair]
            recip = stg.tile([P, SC], F32, tag="recip", name=f"recip_{qc}_{pair}")
            nc.vector.reciprocal(recip[0:DH, :], cx[0:DH, :])
            nc.vector.reciprocal(recip[DH:P, :], cy[DH:P, :])
            nc.vector.tensor_tensor(
                ctx_sb[pair][0:DH, qsl], cy[0:DH, :], recip[DH:P, :], MULT)
            nc.vector.tensor_tensor(
                ctx_sb[pair][DH:P, qsl], cx[DH:P, :], recip[0:DH, :], MULT)
        for it in pending:  # anything not yet drained (shouldn't be much)
            it()
        if qc + 1 < NQC:
            pending = om_items(qc)
        else:
            pending = []
            for it in om_items(qc):
                it()


def build(repeat=1):
    nc = bacc.Bacc("TRN2", target_bir_lowering=False, debug=False,
                   num_devices=NCORES)
    blob = nc.dram_tensor("blob", [P, TOT], F16, kind="ExternalInput").ap()
    outT = nc.dram_tensor("outT", [P, NFT, NQC, SC], F16,
                          kind="ExternalOutput").ap()
    with tile.TileContext(nc) as tc, ExitStack() as cctx:
        hands = _emit_const(cctx, tc, blob)
        pools = _emit_pools(cctx, tc)
        for _ in range(repeat):
            _emit(pools, tc, hands, blob, outT)
    nc.compile()
    return nc


def make_in_maps(query, mask, Wq, Wk, Wv, Wo):
    scale = 1.0 / math.sqrt(DH)
    in_maps = []
    for b in range(B):
        # x section: [P, NQC, NDT, SC]; elem (p, sc, dt, s) = x[sc*SC+s, dt*P+p]
        xt = query[b].T.reshape(NDT, P, NQC, SC).transpose(1, 2, 0, 3)
        xsec = np.ascontiguousarray(xt, dtype=np.float16).reshape(P, NQC * XW)
        # keep section: [P, NQC, NKT, SC]; (p, qc, kt, q) = keep[kt*P+p, qc*SC+q]
        kp = (~mask[b]).T.reshape(NKT, P, NQC, SC).transpose(1, 2, 0, 3)
        ksec = np.ascontiguousarray(kp, dtype=np.float16).reshape(P, NQC * KW)
        for g in range(GROUPS):
            f0 = g * FPC

            def pack_w(wT):  # [D, FPC] -> [P, NDT*FPC] ([p, dt*FPC+f])
                return np.ascontiguousarray(
                    wT.reshape(NDT, P, FPC).transpose(1, 0, 2),
                    dtype=np.float16).reshape(P, WW)

            wosec = np.ascontiguousarray(
                Wo[:, f0:f0 + FPC].T.reshape(FPC // P, P, D).transpose(1, 0, 2),
                dtype=np.float16).reshape(P, OW)
            blob = np.concatenate([
                xsec, ksec,
                pack_w((Wq[f0:f0 + FPC, :] * scale).T),
                pack_w(Wk[f0:f0 + FPC, :].T),
                pack_w(Wv[f0:f0 + FPC, :].T),
                wosec,
            ], axis=1)
            assert blob.shape == (P, TOT) and blob.dtype == np.float16
            in_maps.append({"blob": blob})
    return in_maps


_NC_CACHE = {}


def _get_nc():
    if "nc" not in _NC_CACHE:
        _NC_CACHE["nc"] = build()
    return _NC_CACHE["nc"]


def gather(results, bo):
    out = np.empty((B, S, D), dtype=np.float32)
    for b in range(B):
        acc = results[b * GROUPS]["outT"].astype(np.float32)
        for g in range(1, GROUPS):
            acc = acc + results[b * GROUPS + g]["outT"]
        # [P, NFT, NQC, SC] -> [D, S]: feature f = ft*P + p, pos s = qc*SC + q
        full = acc.transpose(1, 0, 2, 3).reshape(D, S)
        out[b] = full.T + bo.astype(np.float32)
    return out


def kernel(query, mask, Wq, Wk, Wv, Wo, bo, **kwargs):
    nc = _get_nc()
    in_maps = make_in_maps(np.asarray(query), np.asarray(mask), np.asarray(Wq),
                           np.asarray(Wk), np.asarray(Wv), np.asarray(Wo))
    res = run_bass_kernel_spmd(nc, in_maps, list(range(NCORES)))
    return gather(res.results, np.asarray(bo))
